# revision 7
# baseline (speedup 1.0000x reference)
"""Bahdanau attention Trainium2 kernel.

reference math (per batch b):
    z[t, u]  = sum_d feat[t, d] * w1[u, d] + w1_b[u] + (hidden @ w2.T)[u] + w2_b[u]
    score[t] = sum_u v[u] * tanh(z[t, u]) + v_b
    attn     = softmax_t(score)
    ctx[d]   = sum_t attn[t] * feat[t, d]

Sharding: data-parallel over batch, 8 batches per core, params replicated.
Features are uploaded per-core transposed to [NB, 2, 128, T] (d on SBUF
partitions) so the w1 contraction (over d) needs no on-device transpose.

Per core the pipeline is:
  - tiny matmul:  cT[u, b] = w2.T-aug @ hidden-aug  (bias folded via ones row)
  - per (group of 2 batches, 512-wide t-tile):
      PE:  z_psum[128u, 512t] = w1T.T @ featT      (2 u-chunks x 2 d-chunks)
      ACT: tanh_sb = tanh(z_psum + cT[:, b])        (per-partition bias)
      PE:  s_psum[1, 512t]   += vT.T @ tanh_sb      (reduce over u)
      ACT: exp_sb = exp(s_psum + v_b), accum_out -> running sum over t
  - DVE: 1/sum;  ACT: broadcast recip along a [1,128] row
  - PE:  ab_psum[128, 1024] = recip_row.T @ exp_row  (rank-1: attn bcast)
  - DVE: tensor_tensor_reduce(featT * ab_psum) sum over t -> ctx[128d, 1]
"""

import numpy as np

B, T, D, U = 64, 2048, 256, 256
NCORES = 8
NB = B // NCORES  # batches per core
P = 128
GSZ = 2  # batches per softmax group

_BUILD_CACHE = {}


def build_nc(nb=NB, t=T, v_b=0.0, gsz=GSZ):
    """Build the Bass program (same program for all cores)."""
    from contextlib import ExitStack

    import concourse.bass as bass
    import concourse.tile as tile
    from concourse import bacc, mybir

    f32 = mybir.dt.float32
    AF = mybir.ActivationFunctionType
    ALU = mybir.AluOpType

    TGS = 512            # t-tile (moving free dim max)
    ntg = t // TGS       # t-tiles per batch
    H = t // 2           # half-T for the attn-bcast/ctx stage
    ng = nb // gsz       # softmax groups

    nc = bacc.Bacc("TRN2", target_bir_lowering=False, debug=False)

    featT_d = nc.dram_tensor("featT", [nb, 2, P, t], f32, kind="ExternalInput")
    w1T_d = nc.dram_tensor("w1T", [2, P, U], f32, kind="ExternalInput")
    w2T_d = nc.dram_tensor("w2T", [2, P, U], f32, kind="ExternalInput")
    bsum_d = nc.dram_tensor("bsum", [1, U], f32, kind="ExternalInput")
    hT_d = nc.dram_tensor("hT", [2, P, nb], f32, kind="ExternalInput")
    vT_d = nc.dram_tensor("vT", [P, 2], f32, kind="ExternalInput")
    ctx_d = nc.dram_tensor("ctx", [2, P, nb], f32, kind="ExternalOutput")
    attn_d = nc.dram_tensor("attn", [nb, t], f32, kind="ExternalOutput")

    with tile.TileContext(nc) as tc, ExitStack() as es:
        const = es.enter_context(tc.tile_pool(name="const", bufs=1))
        featp = es.enter_context(tc.tile_pool(name="feat", bufs=2 * nb))
        thp = es.enter_context(tc.tile_pool(name="th", bufs=4))
        exp_p = es.enter_context(tc.tile_pool(name="exp", bufs=2))
        smlp = es.enter_context(tc.tile_pool(name="sml", bufs=4))
        rrp = es.enter_context(tc.tile_pool(name="rr", bufs=2))
        scrp = es.enter_context(tc.tile_pool(name="scr", bufs=2))
        cpp = es.enter_context(tc.tile_pool(name="cp", bufs=4))
        zps = es.enter_context(
            tc.tile_pool(name="zps", bufs=2, space=bass.MemorySpace.PSUM))
        sps = es.enter_context(
            tc.tile_pool(name="sps", bufs=2, space=bass.MemorySpace.PSUM))
        abps = es.enter_context(
            tc.tile_pool(name="abps", bufs=2, space=bass.MemorySpace.PSUM))

        # ---- params -> SBUF ------------------------------------------------
        w1sb, w2sb, hsb = [], [], []
        for dc in range(2):
            w1sb.append(const.tile([P, U], f32, tag=f"w1_{dc}", name=f"w1_{dc}"))
            nc.sync.dma_start(w1sb[dc][:], w1T_d[dc])
            w2sb.append(const.tile([P, U], f32, tag=f"w2_{dc}", name=f"w2_{dc}"))
            nc.sync.dma_start(w2sb[dc][:], w2T_d[dc])
            hsb.append(const.tile([P, nb], f32, tag=f"h_{dc}", name=f"h_{dc}"))
            nc.sync.dma_start(hsb[dc][:], hT_d[dc])
        bsumsb = const.tile([1, U], f32, tag="bsum")
        nc.sync.dma_start(bsumsb[:], bsum_d[:])
        vsb = const.tile([P, 2], f32, tag="v")
        nc.sync.dma_start(vsb[:], vT_d[:])

        ones_nb = const.tile([1, nb], f32, tag="ones_nb")
        nc.vector.memset(ones_nb[:], 1.0)
        ones_row = const.tile([1, P], f32, tag="ones_row")
        nc.vector.memset(ones_row[:], 1.0)
        vbias = const.tile([1, 1], f32, tag="vbias")
        nc.vector.memset(vbias[:], float(v_b))

        # ---- features -> SBUF (resident) ----------------------------------
        ft = [[None, None] for _ in range(nb)]
        for b in range(nb):
            for dc in range(2):
                tl = featp.tile([P, t], f32, tag="ft", name=f"ft_{b}_{dc}")
                nc.sync.dma_start(tl[:], featT_d[b, dc])
                ft[b][dc] = tl

        # ---- cT[u, b] = w2T-aug @ hT-aug ----------------------------------
        ctsb = const.tile([P, 2 * nb], f32, tag="ct")
        for uc in range(2):
            cps = zps.tile([P, nb], f32, tag="z")
            nc.tensor.matmul(cps[:], w2sb[0][:, uc * P:(uc + 1) * P], hsb[0][:],
                             start=True, stop=False)
            nc.tensor.matmul(cps[:], w2sb[1][:, uc * P:(uc + 1) * P], hsb[1][:],
                             start=False, stop=False)
            nc.tensor.matmul(cps[:], bsumsb[0:1, uc * P:(uc + 1) * P],
                             ones_nb[:], start=False, stop=True)
            nc.vector.tensor_copy(ctsb[:, uc * nb:(uc + 1) * nb], cps[:])

        ctxsb = const.tile([P, 2 * nb], f32, tag="ctx")

        # ---- main loop over batches ---------------------------------------
        def phase_a(b):
            ex = exp_p.tile([1, t], f32, tag="ex", name=f"ex_{b}")
            sa = smlp.tile([1, ntg], f32, tag="sa", name=f"sa_{b}")
            for tg in range(ntg):
                tsl = slice(tg * TGS, (tg + 1) * TGS)
                stile = sps.tile([1, TGS], f32, tag="s", name=f"s_{b}_{tg}")
                ths = []
                for uc in range(2):
                    zt = zps.tile([P, TGS], f32, tag="z", name=f"z_{b}_{tg}_{uc}")
                    nc.tensor.matmul(
                        zt[:], w1sb[0][:, uc * P:(uc + 1) * P],
                        ft[b][0][:, tsl], start=True, stop=False)
                    nc.tensor.matmul(
                        zt[:], w1sb[1][:, uc * P:(uc + 1) * P],
                        ft[b][1][:, tsl], start=False, stop=True)
                    th = thp.tile([P, TGS], f32, tag="th", name=f"th_{b}_{tg}_{uc}")
                    nc.scalar.activation(
                        th[:], zt[:], AF.Tanh,
                        bias=ctsb[:, uc * nb + b:uc * nb + b + 1])
                    ths.append(th)
                for uc in range(2):
                    nc.tensor.matmul(
                        stile[0:1, :], vsb[:, uc:uc + 1], ths[uc][:],
                        start=(uc == 0), stop=(uc == 1))
                # fused exp + running sum over this t-tile
                nc.scalar.activation(
                    ex[:, tsl], stile[:], AF.Exp, bias=vbias[:],
                    accum_out=sa[:, tg:tg + 1])
            se = smlp.tile([1, 1], f32, tag="se", name=f"se_{b}")
            nc.vector.reduce_sum(se[:], sa[:], axis=mybir.AxisListType.X)
            rec = smlp.tile([1, 1], f32, tag="rec", name=f"rec_{b}")
            nc.vector.reciprocal(rec[:], se[:])
            return ex, rec

        def phase_c(b, ex, rec):
            asb = rrp.tile([1, t], f32, tag="asb", name=f"asb_{b}")
            nc.vector.tensor_scalar_mul(asb[:], ex[:], rec[0:1, 0:1])
            nc.sync.dma_start(attn_d[b:b + 1, :], asb[:])
            cp0 = cpp.tile([P, 1], f32, tag="cp0", name=f"cp0_{b}")
            cp1 = cpp.tile([P, 1], f32, tag="cp1", name=f"cp1_{b}")
            for h in range(2):
                hsl = slice(h * H, (h + 1) * H)
                ab = abps.tile([P, H], f32, tag="ab", name=f"ab_{b}_{h}")
                for q0 in range(0, H, TGS):
                    qn = min(TGS, H - q0)
                    nc.tensor.matmul(
                        ab[:, q0:q0 + qn], ones_row[:],
                        asb[0:1, h * H + q0:h * H + q0 + qn],
                        start=True, stop=True)
                for dc in range(2):
                    scr = scrp.tile([P, H], f32, tag="scr", name=f"scr_{b}_{h}_{dc}")
                    cpx = [cp0, cp1][dc]
                    acc = (cpx[:] if h == 0
                           else ctxsb[:, dc * nb + b:dc * nb + b + 1])
                    nc.vector.scalar_tensor_tensor(
                        out=scr[:], in0=ft[b][dc][:, hsl], scalar=1.0,
                        in1=ab[:], op0=ALU.mult, op1=ALU.mult, accum_out=acc)
            for dc in range(2):
                cpx = [cp0, cp1][dc]
                dst = ctxsb[:, dc * nb + b:dc * nb + b + 1]
                nc.vector.tensor_tensor(out=dst, in0=dst, in1=cpx[:],
                                        op=ALU.add)

        prev = None
        for b in range(nb):
            cur = phase_a(b)
            if prev is not None:
                phase_c(b - 1, *prev)
            prev = cur
        phase_c(nb - 1, *prev)

        for dc in range(2):
            nc.sync.dma_start(ctx_d[dc], ctxsb[:, dc * nb:(dc + 1) * nb])

    nc.compile()
    return nc


def prep_core_inputs(features_c, hidden_c, w1_w, w1_b, w2_w, w2_b, v_w):
    """Host-side layout prep for one core's shard (layout transforms only)."""
    nb = features_c.shape[0]
    featT = np.ascontiguousarray(features_c.transpose(0, 2, 1)).reshape(
        nb, 2, P, -1)
    w1T = np.ascontiguousarray(w1_w.T).reshape(2, P, U)
    w2T = np.ascontiguousarray(w2_w.T).reshape(2, P, U)
    bsum = (w1_b + w2_b).reshape(1, U).astype(np.float32)
    hT = np.ascontiguousarray(hidden_c.T).reshape(2, P, nb)
    vT = np.ascontiguousarray(v_w.reshape(2, P).T)
    return {
        "featT": featT.astype(np.float32),
        "w1T": w1T.astype(np.float32),
        "w2T": w2T.astype(np.float32),
        "bsum": bsum,
        "hT": hT.astype(np.float32),
        "vT": vT.astype(np.float32),
    }


def kernel(features, hidden, w1_w, w1_b, w2_w, w2_b, v_w, v_b, _trace=False):
    from concourse.bass_utils import run_bass_kernel_spmd

    features = np.asarray(features, dtype=np.float32)
    hidden = np.asarray(hidden, dtype=np.float32)
    w1_w = np.asarray(w1_w, dtype=np.float32)
    w1_b = np.asarray(w1_b, dtype=np.float32)
    w2_w = np.asarray(w2_w, dtype=np.float32)
    w2_b = np.asarray(w2_b, dtype=np.float32)
    v_w = np.asarray(v_w, dtype=np.float32)
    vb = float(np.asarray(v_b).reshape(-1)[0])

    key = ("full", NB, T, vb)
    if key not in _BUILD_CACHE:
        _BUILD_CACHE[key] = build_nc(NB, T, vb, GSZ)
    nc = _BUILD_CACHE[key]

    in_maps = []
    for c in range(NCORES):
        sl = slice(c * NB, (c + 1) * NB)
        in_maps.append(prep_core_inputs(
            features[sl], hidden[sl], w1_w, w1_b, w2_w, w2_b, v_w))

    res = run_bass_kernel_spmd(nc, in_maps, list(range(NCORES)), trace=_trace)

    context = np.empty((B, D), dtype=np.float32)
    attn = np.empty((B, T, 1), dtype=np.float32)
    for c in range(NCORES):
        r = res.results[c]
        # ctx [2, 128, nb] -> [nb, 256]
        context[c * NB:(c + 1) * NB] = (
            r["ctx"].transpose(2, 0, 1).reshape(NB, D))
        attn[c * NB:(c + 1) * NB] = r["attn"][..., None]
    kernel._last_exec_ns = res.exec_time_ns
    kernel._last_results = res
    return context, attn


# revision 9
# speedup vs baseline: 2.3210x; 2.3210x over previous
"""Bahdanau attention Trainium2 kernel.

reference math (per batch b):
    z[t, u]  = sum_d feat[t, d] * w1[u, d] + w1_b[u] + (hidden @ w2.T)[u] + w2_b[u]
    score[t] = sum_u v[u] * tanh(z[t, u]) + v_b
    attn     = softmax_t(score)
    ctx[d]   = sum_t attn[t] * feat[t, d]

Sharding: data-parallel over batch, 8 batches per core, params replicated.
Features are uploaded per-core transposed to [NB, 2, 128, T] (d on SBUF
partitions) so the w1 contraction (over d) needs no on-device transpose,
and cast to bf16 (fp32 matmuls run at half PE rate via LOW_HIGH two-pass
mode and fp32 LDWEIGHTS can't use fast-weight-load; bf16 also halves HBM
traffic). PSUM accumulation stays fp32, and the softmax chain
(exp/sum/reciprocal/normalize) runs in fp32.

Per core the pipeline is:
  - tiny fp32 matmul: cT[u, b] = w2.T-aug @ hidden-aug (biases folded in)
  - per (batch, 1024-wide t-tile):
      PE:  z_psum[128u, 1024t] = w1T.T @ featT        (bf16, 2 u x 2 d)
      ACT: tanh_sb(bf16) = tanh(z_psum + cT[:, b])    (per-partition bias)
      PE:  s_psum[1, 1024t] += vT.T @ tanh_sb          (reduce over u)
      ACT: ex(f32) = exp(s_psum + v_b), accum_out -> running sum over t
  - DVE: rec = 1/sum; attn = ex * rec (f32, the attn output) + bf16 copy
  - PE:  ab_psum[128, 1024] = ones.T @ attn_bf16       (rank-1 broadcast)
  - DVE: scalar_tensor_tensor(ft * ab) accum -> ctx[128d, 1] per d-chunk
"""

import numpy as np

B, T, D, U = 64, 2048, 256, 256
NCORES = 8
NB = B // NCORES  # batches per core
P = 128

_BUILD_CACHE = {}


def build_nc(nb=NB, t=T, v_b=0.0):
    """Build the Bass program (same program for all cores)."""
    from contextlib import ExitStack

    import concourse.bass as bass
    import concourse.tile as tile
    from concourse import bacc, mybir

    f32 = mybir.dt.float32
    bf16 = mybir.dt.bfloat16
    AF = mybir.ActivationFunctionType
    ALU = mybir.AluOpType

    ST = min(1024, t)    # t super-tile (z/tanh/score/exp granularity)
    nst = t // ST
    H = t // 2           # half-T for the attn-bcast/ctx stage
    MF = 512             # max moving free dim per fp32-psum-bank matmul

    nc = bacc.Bacc("TRN2", target_bir_lowering=False, debug=False)

    featT_d = nc.dram_tensor("featT", [nb, 2, P, t], bf16, kind="ExternalInput")
    w1T_d = nc.dram_tensor("w1T", [2, P, U], bf16, kind="ExternalInput")
    w2T_d = nc.dram_tensor("w2T", [2, P, U], f32, kind="ExternalInput")
    bsum_d = nc.dram_tensor("bsum", [1, U], f32, kind="ExternalInput")
    hT_d = nc.dram_tensor("hT", [2, P, nb], f32, kind="ExternalInput")
    vT_d = nc.dram_tensor("vT", [P, 2], bf16, kind="ExternalInput")
    ctx_d = nc.dram_tensor("ctx", [2, P, nb], f32, kind="ExternalOutput")
    attn_d = nc.dram_tensor("attn", [nb, t], f32, kind="ExternalOutput")

    with tile.TileContext(nc) as tc, ExitStack() as es:
        const = es.enter_context(tc.tile_pool(name="const", bufs=1))
        featp = es.enter_context(tc.tile_pool(name="feat", bufs=2 * nb))
        thp = es.enter_context(tc.tile_pool(name="th", bufs=4))
        exp_p = es.enter_context(tc.tile_pool(name="exp", bufs=3))
        smlp = es.enter_context(tc.tile_pool(name="sml", bufs=4))
        rrp = es.enter_context(tc.tile_pool(name="rr", bufs=2))
        scrp = es.enter_context(tc.tile_pool(name="scr", bufs=2))
        cpp = es.enter_context(tc.tile_pool(name="cp", bufs=4))
        zps = es.enter_context(
            tc.tile_pool(name="zps", bufs=2, space=bass.MemorySpace.PSUM))
        sps = es.enter_context(
            tc.tile_pool(name="sps", bufs=1, space=bass.MemorySpace.PSUM))
        abps = es.enter_context(
            tc.tile_pool(name="abps", bufs=1, space=bass.MemorySpace.PSUM))

        # ---- params -> SBUF ------------------------------------------------
        w1sb, w2sb, hsb = [], [], []
        for dc in range(2):
            w1sb.append(const.tile([P, U], bf16, tag=f"w1_{dc}",
                                   name=f"w1_{dc}"))
            nc.sync.dma_start(w1sb[dc][:], w1T_d[dc])
            w2sb.append(const.tile([P, U], f32, tag=f"w2_{dc}",
                                   name=f"w2_{dc}"))
            nc.sync.dma_start(w2sb[dc][:], w2T_d[dc])
            hsb.append(const.tile([P, nb], f32, tag=f"h_{dc}", name=f"h_{dc}"))
            nc.sync.dma_start(hsb[dc][:], hT_d[dc])
        bsumsb = const.tile([1, U], f32, tag="bsum")
        nc.sync.dma_start(bsumsb[:], bsum_d[:])
        vsb = const.tile([P, 2], bf16, tag="v")
        nc.sync.dma_start(vsb[:], vT_d[:])

        ones_nb = const.tile([1, nb], f32, tag="ones_nb")
        nc.vector.memset(ones_nb[:], 1.0)
        ones_row = const.tile([1, P], bf16, tag="ones_row")
        nc.vector.memset(ones_row[:], 1.0)
        vbias = const.tile([1, 1], f32, tag="vbias")
        nc.vector.memset(vbias[:], float(v_b))

        # ---- features -> SBUF (resident) ----------------------------------
        ft = [[None, None] for _ in range(nb)]
        for b in range(nb):
            for dc in range(2):
                tl = featp.tile([P, t], bf16, tag="ft", name=f"ft_{b}_{dc}")
                nc.sync.dma_start(tl[:], featT_d[b, dc])
                ft[b][dc] = tl

        # ---- cT[u, b] = w2T-aug @ hT-aug (fp32, exact) --------------------
        ctsb = const.tile([P, 2 * nb], f32, tag="ct")
        for uc in range(2):
            cps = zps.tile([P, nb], f32, tag="z", name=f"cps_{uc}")
            nc.tensor.matmul(cps[:], w2sb[0][:, uc * P:(uc + 1) * P], hsb[0][:],
                             start=True, stop=False)
            nc.tensor.matmul(cps[:], w2sb[1][:, uc * P:(uc + 1) * P], hsb[1][:],
                             start=False, stop=False)
            nc.tensor.matmul(cps[:], bsumsb[0:1, uc * P:(uc + 1) * P],
                             ones_nb[:], start=False, stop=True)
            nc.vector.tensor_copy(ctsb[:, uc * nb:(uc + 1) * nb], cps[:])

        ctxsb = const.tile([P, 2 * nb], f32, tag="ctx")

        # ---- main loop over batches ---------------------------------------
        def phase_a(b):
            ex = exp_p.tile([1, t], f32, tag="ex", name=f"ex_{b}")
            sa = smlp.tile([1, nst], f32, tag="sa", name=f"sa_{b}")
            for st in range(nst):
                s0 = st * ST
                ths = []
                for uc in range(2):
                    zt = zps.tile([P, ST], f32, tag="z", name=f"z_{b}_{st}_{uc}")
                    for q0 in range(0, ST, MF):
                        for dc in range(2):
                            nc.tensor.matmul(
                                zt[:, q0:q0 + MF],
                                w1sb[dc][:, uc * P:(uc + 1) * P],
                                ft[b][dc][:, s0 + q0:s0 + q0 + MF],
                                start=(dc == 0), stop=(dc == 1))
                    th = thp.tile([P, ST], bf16, tag="th",
                                  name=f"th_{b}_{st}_{uc}")
                    nc.scalar.activation(
                        th[:], zt[:], AF.Tanh,
                        bias=ctsb[:, uc * nb + b:uc * nb + b + 1])
                    ths.append(th)
                stile = sps.tile([1, ST], f32, tag="s", name=f"s_{b}_{st}")
                for q0 in range(0, ST, MF):
                    for uc in range(2):
                        nc.tensor.matmul(
                            stile[0:1, q0:q0 + MF], vsb[:, uc:uc + 1],
                            ths[uc][:, q0:q0 + MF],
                            start=(uc == 0), stop=(uc == 1))
                # fused exp + running sum over this t-tile
                nc.scalar.activation(
                    ex[:, s0:s0 + ST], stile[:], AF.Exp, bias=vbias[:],
                    accum_out=sa[:, st:st + 1])
            se = smlp.tile([1, 1], f32, tag="se", name=f"se_{b}")
            nc.vector.reduce_sum(se[:], sa[:], axis=mybir.AxisListType.X)
            rec = smlp.tile([1, 1], f32, tag="rec", name=f"rec_{b}")
            nc.vector.reciprocal(rec[:], se[:])
            return ex, rec

        def phase_c(b, ex, rec):
            asb = rrp.tile([1, t], f32, tag="asb", name=f"asb_{b}")
            nc.vector.tensor_scalar_mul(asb[:], ex[:], rec[0:1, 0:1])
            nc.sync.dma_start(attn_d[b:b + 1, :], asb[:])
            abf = rrp.tile([1, t], bf16, tag="abf", name=f"abf_{b}")
            nc.vector.tensor_copy(abf[:], asb[:])
            cp0 = cpp.tile([P, 1], f32, tag="cp0", name=f"cp0_{b}")
            cp1 = cpp.tile([P, 1], f32, tag="cp1", name=f"cp1_{b}")
            for h in range(2):
                hsl = slice(h * H, (h + 1) * H)
                ab = abps.tile([P, H], f32, tag="ab", name=f"ab_{b}_{h}")
                for q0 in range(0, H, 512):
                    qn = min(512, H - q0)
                    nc.tensor.matmul(
                        ab[:, q0:q0 + qn], ones_row[:],
                        abf[0:1, h * H + q0:h * H + q0 + qn],
                        start=True, stop=True)
                for dc in range(2):
                    scr = scrp.tile([P, H], f32, tag="scr",
                                    name=f"scr_{b}_{h}_{dc}")
                    cpx = [cp0, cp1][dc]
                    acc = (cpx[:] if h == 0
                           else ctxsb[:, dc * nb + b:dc * nb + b + 1])
                    nc.vector.scalar_tensor_tensor(
                        out=scr[:], in0=ft[b][dc][:, hsl], scalar=1.0,
                        in1=ab[:], op0=ALU.mult, op1=ALU.mult, accum_out=acc)
            for dc in range(2):
                cpx = [cp0, cp1][dc]
                dst = ctxsb[:, dc * nb + b:dc * nb + b + 1]
                nc.vector.tensor_tensor(out=dst, in0=dst, in1=cpx[:],
                                        op=ALU.add)

        prev = None
        for b in range(nb):
            cur = phase_a(b)
            if prev is not None:
                phase_c(b - 1, *prev)
            prev = cur
        phase_c(nb - 1, *prev)

        for dc in range(2):
            nc.sync.dma_start(ctx_d[dc], ctxsb[:, dc * nb:(dc + 1) * nb])

    nc.compile()
    return nc


def prep_core_inputs(features_c, hidden_c, w1_w, w1_b, w2_w, w2_b, v_w):
    """Host-side layout prep for one core's shard (layout/dtype transforms)."""
    import ml_dtypes

    bf16 = ml_dtypes.bfloat16
    nb = features_c.shape[0]
    featT = np.ascontiguousarray(features_c.transpose(0, 2, 1)).reshape(
        nb, 2, P, -1)
    w1T = np.ascontiguousarray(w1_w.T).reshape(2, P, U)
    w2T = np.ascontiguousarray(w2_w.T).reshape(2, P, U)
    bsum = (w1_b + w2_b).reshape(1, U).astype(np.float32)
    hT = np.ascontiguousarray(hidden_c.T).reshape(2, P, nb)
    vT = np.ascontiguousarray(v_w.reshape(2, P).T)
    return {
        "featT": featT.astype(bf16),
        "w1T": w1T.astype(bf16),
        "w2T": w2T.astype(np.float32),
        "bsum": bsum,
        "hT": hT.astype(np.float32),
        "vT": vT.astype(bf16),
    }


def kernel(features, hidden, w1_w, w1_b, w2_w, w2_b, v_w, v_b, _trace=False):
    from concourse.bass_utils import run_bass_kernel_spmd

    features = np.asarray(features, dtype=np.float32)
    hidden = np.asarray(hidden, dtype=np.float32)
    w1_w = np.asarray(w1_w, dtype=np.float32)
    w1_b = np.asarray(w1_b, dtype=np.float32)
    w2_w = np.asarray(w2_w, dtype=np.float32)
    w2_b = np.asarray(w2_b, dtype=np.float32)
    v_w = np.asarray(v_w, dtype=np.float32)
    vb = float(np.asarray(v_b).reshape(-1)[0])

    key = ("full", NB, T, vb)
    if key not in _BUILD_CACHE:
        _BUILD_CACHE[key] = build_nc(NB, T, vb)
    nc = _BUILD_CACHE[key]

    in_maps = []
    for c in range(NCORES):
        sl = slice(c * NB, (c + 1) * NB)
        in_maps.append(prep_core_inputs(
            features[sl], hidden[sl], w1_w, w1_b, w2_w, w2_b, v_w))

    res = run_bass_kernel_spmd(nc, in_maps, list(range(NCORES)), trace=_trace)

    context = np.empty((B, D), dtype=np.float32)
    attn = np.empty((B, T, 1), dtype=np.float32)
    for c in range(NCORES):
        r = res.results[c]
        # ctx [2, 128, nb] -> [nb, 256]
        context[c * NB:(c + 1) * NB] = (
            r["ctx"].transpose(2, 0, 1).reshape(NB, D))
        attn[c * NB:(c + 1) * NB] = r["attn"][..., None]
    kernel._last_exec_ns = res.exec_time_ns
    kernel._last_results = res
    return context, attn
